# revision 10
# baseline (speedup 1.0000x reference)
"""Data-dependent ALiBi bias kernel for Trainium2, distributed over 8 NeuronCores.

Reference computation (per full input):
    logits = einsum('bnd,hd->bhn', x, W) + b          # [2, 16, 2048]
    fg     = log_sigmoid(logits)                      # [2, 16, 2048]
    fg     = cumsum(fg, axis=-1)
    out    = fg[:, :, :, None] - fg[:, :, None, :]    # [2, 16, 2048, 2048]

Sharding: 32 (batch, head) pairs / 8 cores = 4 heads per core, batch-major
(cores 0-3 take batch 0, cores 4-7 take batch 1). Each core computes its own
[4, 2048, 2048] slab independently; no collectives.

Device algorithm per core (all f32):
    1. logits^T [4, n] via PE matmul of host-pre-transposed x^T with W^T
    2. u = ln(1 + exp(-(logits + b)))   (= -log_sigmoid(logits), via ACT)
    3. g = cumsum(u)                    (DVE tensor_tensor_scan; g = -fg_cum)
    4. out[h, i, j] = fg_cum[i] - fg_cum[j] = g[j] - g[i]:
       g broadcast across partitions (ones-matmul) gives the j-term,
       PE-transposed negated g columns give the per-partition i-term bias,
       one ACT Identity per [128, 2048] tile + DMA out.
"""

import numpy as np

B = 2
NH = 16
N = 2048
D = 1024
NCORES = 8
HPC = (B * NH) // NCORES  # 4 (batch, head) pairs per core
P = 128
DC = D // P    # 8 contraction chunks
NCH = N // P   # 16 row chunks per head
NMM = 512      # matmul moving free dim
NJ = N // NMM  # 4

_CACHE = {}


def _build_nc():
    import concourse.bacc as bacc
    import concourse.mybir as mybir
    from concourse.masks import make_identity
    from concourse.tile import TileContext

    f32 = mybir.dt.float32
    Act = mybir.ActivationFunctionType
    nc = bacc.Bacc(None, target_bir_lowering=False)

    xT = nc.dram_tensor("xT", [D, N], f32, kind="ExternalInput")
    Wt = nc.dram_tensor("Wt", [D, HPC], f32, kind="ExternalInput")
    bv = nc.dram_tensor("bv", [HPC, 1], f32, kind="ExternalInput")
    out = nc.dram_tensor("out", [HPC, N, N], f32, kind="ExternalOutput")

    with TileContext(nc) as tc:
        with (
            tc.tile_pool(name="big", bufs=1) as big,
            tc.tile_pool(name="small", bufs=1) as small,
            tc.tile_pool(name="lps", bufs=2, space="PSUM") as lps,
            tc.tile_pool(name="gps", bufs=2, space="PSUM") as gps,
            tc.tile_pool(name="bps", bufs=1, space="PSUM") as bps,
            tc.tile_pool(name="grp", bufs=2) as grp,
            tc.tile_pool(name="outp", bufs=6) as outp,
        ):
            # ---- inputs -> SBUF
            xT_s = big.tile([P, DC, N], f32, tag="xT")
            xT_r = xT.rearrange("(c p) n -> p c n", p=P)
            for j in range(NJ):
                nc.sync.dma_start(
                    out=xT_s[:, :, j * NMM : (j + 1) * NMM],
                    in_=xT_r[:, :, j * NMM : (j + 1) * NMM],
                )
            Wt_s = small.tile([P, DC, HPC], f32, tag="Wt")
            nc.sync.dma_start(out=Wt_s, in_=Wt.rearrange("(c p) h -> p c h", p=P))
            b_s = small.tile([HPC, 1], f32, tag="b")
            nc.sync.dma_start(out=b_s, in_=bv[:])
            nb = small.tile([HPC, 1], f32, tag="nb")
            nc.vector.tensor_scalar_mul(nb, b_s, -1.0)

            ones1 = small.tile([1, P], f32, tag="ones1")
            nc.gpsimd.memset(ones1, 1.0)
            ident = small.tile([HPC, HPC], f32, tag="ident")
            make_identity(nc, ident)
            zeros = small.tile([HPC, N], f32, tag="zeros")
            nc.gpsimd.memset(zeros, 0.0)

            t_exp = small.tile([HPC, N], f32, tag="t_exp")
            g = small.tile([HPC, N], f32, tag="g")
            ngcol = small.tile([P, NCH * HPC], f32, tag="ngcol")
            bcast = big.tile([P, HPC, N], f32, tag="bcast")

            # ---- logits^T [4, n] ; t = exp(-(logits + b))
            for j in range(NJ):
                ps = lps.tile([HPC, NMM], f32, tag="lps")
                for c in range(DC):
                    nc.tensor.matmul(
                        ps,
                        Wt_s[:, c, :],
                        xT_s[:, c, j * NMM : (j + 1) * NMM],
                        start=(c == 0),
                        stop=(c == DC - 1),
                    )
                nc.scalar.activation(
                    t_exp[:, j * NMM : (j + 1) * NMM],
                    ps,
                    Act.Exp,
                    bias=nb[:, 0:1],
                    scale=-1.0,
                )
            # u = ln(1 + t) = -log_sigmoid(logits)  (in place on t_exp)
            nc.scalar.activation(t_exp, t_exp, Act.Ln, bias=1.0)
            # g = cumsum(u)
            nc.vector.tensor_tensor_scan(
                g, t_exp, zeros, 0.0, mybir.AluOpType.add, mybir.AluOpType.add
            )

            # ---- negated g columns: ngcol[p, c*HPC + h] = -g[h, c*P + p]
            for c in range(NCH):
                gp = gps.tile([P, HPC], f32, tag="gps")
                nc.tensor.transpose(gp, g[:, c * P : (c + 1) * P], ident)
                nc.vector.tensor_scalar_mul(
                    ngcol[:, c * HPC : (c + 1) * HPC], gp, -1.0
                )

            # ---- bcast[p, h, j] = g[h, j] for all p
            # (g row first DMA'd to partition 0: PE matmul needs base partition 0)
            for h in range(HPC):
                grow = grp.tile([1, N], f32, tag="grow")
                nc.sync.dma_start(out=grow, in_=g[h : h + 1, :])
                bp = bps.tile([P, N], f32, tag="bps")
                for j in range(NJ):
                    nc.tensor.matmul(
                        bp[:, j * NMM : (j + 1) * NMM],
                        ones1,
                        grow[:, j * NMM : (j + 1) * NMM],
                    )
                nc.vector.tensor_copy(bcast[:, h, :], bp)

            # ---- out[h, c*P + p, :] = g[:] - g[h, c*P + p]
            for h in range(HPC):
                for c in range(NCH):
                    ot = outp.tile([P, N], f32, tag="ot")
                    col = c * HPC + h
                    nc.scalar.activation(
                        ot,
                        bcast[:, h, :],
                        Act.Identity,
                        bias=ngcol[:, col : col + 1],
                        scale=1.0,
                    )
                    nc.sync.dma_start(out=out[h, c * P : (c + 1) * P, :], in_=ot)

    if not nc.is_finalized():
        nc.finalize()
    return nc


def _get_nc():
    if "nc" not in _CACHE:
        _CACHE["nc"] = _build_nc()
    return _CACHE["nc"]


def _make_in_maps(x, W, b):
    x = np.ascontiguousarray(x, dtype=np.float32)
    W = np.ascontiguousarray(W, dtype=np.float32)
    b = np.ascontiguousarray(b, dtype=np.float32)
    xT_by_batch = [np.ascontiguousarray(x[bi].T) for bi in range(B)]
    in_maps = []
    for k in range(NCORES):
        bi = k // (NCORES // B)
        h0 = (k % (NCORES // B)) * HPC
        in_maps.append(
            {
                "xT": xT_by_batch[bi],
                "Wt": np.ascontiguousarray(W[h0 : h0 + HPC].T),
                "bv": np.ascontiguousarray(b[h0 : h0 + HPC].reshape(HPC, 1)),
            }
        )
    return in_maps


def kernel(x, W, b, _trace=False):
    from concourse.bass_utils import run_bass_kernel_spmd

    nc = _get_nc()
    in_maps = _make_in_maps(x, W, b)
    res = run_bass_kernel_spmd(
        nc, in_maps, core_ids=list(range(NCORES)), trace=_trace
    )
    _CACHE["last_results"] = res
    full = np.empty((B, NH, N, N), dtype=np.float32)
    for k in range(NCORES):
        bi = k // (NCORES // B)
        h0 = (k % (NCORES // B)) * HPC
        full[bi, h0 : h0 + HPC] = res.results[k]["out"]
    return full
